# revision 1
# baseline (speedup 1.0000x reference)
"""MultiHeadSelectiveAttention TRN2 kernel: full inputs -> full output.

Shards batch (B=8) across 8 NeuronCores (data parallel, one batch element
per core). Each core runs a Bass/Tile kernel computing, for its batch b:
    v = x Wv + bv;  xv = x^T v;  ktv = blockdiag(Wk^T xv + bk (x) sum(v));
    U = Wq ktv; c = bq^T ktv;  out = sigmoid((x U + c)/8)^T * mask
which is mathematically identical to the reference attention (value head
dim 1 makes the L x L score matrix collapse).
"""
import sys, os
sys.path.insert(0, '/opt/trn_rl_repo')
sys.path.insert(0, os.path.dirname(os.path.abspath(__file__)))
import numpy as np

B, L, D, H = 8, 4096, 1024, 16

_cache = {}

def _get_nc():
    if "nc" not in _cache:
        from kernel_core import build
        _cache["nc"] = build()[0]
    return _cache["nc"]


def kernel(x, mask, Wq, bq, Wk, bk, Wv, bv):
    from concourse.bass_utils import run_bass_kernel_spmd
    x = np.asarray(x, dtype=np.float32)
    mask_f = np.asarray(mask).astype(np.float32)
    Wq = np.asarray(Wq, dtype=np.float32)
    Wk = np.asarray(Wk, dtype=np.float32)
    Wv = np.asarray(Wv, dtype=np.float32)
    bq = np.asarray(bq, dtype=np.float32)
    bk = np.asarray(bk, dtype=np.float32)
    bv = np.asarray(bv, dtype=np.float32)
    nc = _get_nc()
    bk2 = np.ascontiguousarray(np.broadcast_to(bk[None, :], (H, D)))
    bv2 = np.ascontiguousarray(np.broadcast_to(bv[None, :], (128, H)))
    in_maps = []
    for b in range(B):
        in_maps.append({
            "x": np.ascontiguousarray(x[b]),
            "wq": Wq, "wk": Wk, "wv": Wv,
            "bq": np.ascontiguousarray(bq.reshape(D, 1)),
            "bk": bk2, "bv": bv2,
            "bvc": np.ascontiguousarray(bv.reshape(H, 1)),
            "mk": np.ascontiguousarray(
                np.broadcast_to(mask_f[b][None, :], (H, L))),
        })
    res = run_bass_kernel_spmd(nc, in_maps, core_ids=list(range(B)))
    out = np.stack([res.results[b]["out"] for b in range(B)], axis=0)
    return out.astype(np.float32)
